# revision 29
# baseline (speedup 1.0000x reference)
"""Trainium2 Bass kernel for nn_MAD_72679436582977 (retrieval_knn).

For each edge endpoint (src/tgt of 1024 edges) and each of 4 heads: find the
8 nearest neighbors (excluding self) among 50000 nodes in a 32-d embedding
space, logits (q - e_k).f_q, dist |q - e_k|, softmax(1 - dist) over
16 neighbors + 8 sentinels, sigmoid of head-mean weighted sum.

Strategy: data-parallel over edges across 8 cores (128 edges/core, SPMD, no
collectives).  The distance GEMM s[q, n] = 2 q.e_n - |e_n|^2 runs in f32r
(full PE rate).  Candidate selection is a multi-engine reduction pipeline:

  - per 2048-node super: Act casts PSUM fp32 -> SBUF fp16; a 3-round
    halves max-fold (DVE / GpSimd, fp16 2x mode) pools groups of 8
    {f, f+256, ..., f+1792} down to 256 slots;
  - DVE max8 + max_index on the pooled 256 -> per-super top-8 group
    values + slots (dup-safe hardware semantics);
  - per m-tile: quantize values, pack (qv*8192 + globalslot) into fp32
    ints, top-16 via max8/match_replace/max8 (ties impossible: slot in
    low bits), decode slots -> 16 groups x 8 node gids;
  - ONE batched indirect DMA gathers all 128 candidate rows
    [embed(32) | norm | pad]; exact fp32 recompute s = 2 q.e - |e|^2
    (products on GpSimd, reduce on DVE); exact top-9, drop rank-1
    (self), winner gids via mask*(gid+1) max8;
  - second tiny gather of the 8 winners -> field dots, dist, weights,
    softmax-ratio with sentinel mass, head mean, sigmoid.

Containment of the true top-9 in the 16 gathered groups was validated
against the reference data (0/8192 failures, worst needed rank 14).
"""
import os
import sys

sys.path.insert(0, "/opt/trn_rl_repo")

import numpy as np

import concourse.bass as bass
import concourse.bacc as bacc
import concourse.mybir as mybir
from concourse import tile
from concourse.bass import IndirectOffsetOnAxis

F32 = mybir.dt.float32
F32R = mybir.dt.float32r
F16 = mybir.dt.float16
BF16 = mybir.dt.bfloat16
U32 = mybir.dt.uint32

N_HEADS = 4
N_NODES = 50000
DIM = 32
N_BATCH = 1024
N_SENT = 8
N_CORES = 8

EDGES_PER_CORE = N_BATCH // N_CORES          # 128
SUP = 2048                                    # super width (4 PSUM banks)
N_SUPERS = 25
N_PAD = SUP * N_SUPERS                        # 51200
M_TILES = N_HEADS * 2                         # (head, src/tgt) tiles
KC = DIM + 2                                  # contraction: 32 dims + en_hi + en_lo
N_PAIRS = 13                                  # super-pair scan windows
N_CAND = N_PAIRS * 8                          # 104 group candidates per row
N_WIN = 16                                    # winner groups kept
N_GATH = N_WIN * 8                            # 128 gathered candidate rows
EW = DIM + 2                                  # gathered row: embed|norm|pad
AUXW = 2 + DIM + DIM                          # qn | qf | f | q

TWO23 = 12582912.0   # 1.5*2^23: round-to-int magic (ulp=1 across the sum)

# per-super fold engine assignment: 'G' = all folds on GpSimd,
# 'M' = fold1 DVE + folds2,3 GpSimd, 'D' = all folds on DVE (from PSUM,
# no Act cast).  Tuned for Act/DVE/GpSimd balance.
SUPER_TYPE = list("MMMMM MMMMM MMMMM MMMMM MMMMM".replace(" ", ""))
assert len(SUPER_TYPE) == N_SUPERS

LAST = {}


def _build_program(debug=False):
    nc = bacc.Bacc(None, num_swdge_queues=2)

    rhs_d = nc.dram_tensor("rhs_aug", [N_HEADS, KC, N_PAD], BF16,
                           kind="ExternalInput")
    embn_d = nc.dram_tensor("embn", [N_HEADS * N_PAD, EW], F32,
                            kind="ExternalInput")
    # group-major table: row (h, j, f) = the 8 group members' [emb|en|pad]
    embg_d = nc.dram_tensor("embg", [N_HEADS * N_SUPERS * 256, 8 * EW], F32,
                            kind="ExternalInput")
    qpack_d = nc.dram_tensor("qpack", [M_TILES, KC, 128], BF16,
                             kind="ExternalInput")
    aux_d = nc.dram_tensor("aux", [M_TILES, 128, AUXW], F32,
                           kind="ExternalInput")

    preds_d = nc.dram_tensor("preds", [128, 1], F32, kind="ExternalOutput")
    dbg_gid_d = nc.dram_tensor("dbg_gid", [M_TILES, 128, 8], U32,
                               kind="ExternalOutput")
    dbg_s_d = nc.dram_tensor("dbg_s", [M_TILES, 128, 8], F32,
                             kind="ExternalOutput")
    if debug:
        dbg_cv_d = nc.dram_tensor("dbg_cv", [128, N_CAND], F32,
                                  kind="ExternalOutput")
        dbg_cs_d = nc.dram_tensor("dbg_cs", [128, N_CAND], U32,
                                  kind="ExternalOutput")
        dbg_pk_d = nc.dram_tensor("dbg_pk", [128, N_CAND], F32,
                                  kind="ExternalOutput")
        dbg_w16_d = nc.dram_tensor("dbg_w16", [128, N_WIN], F32,
                                   kind="ExternalOutput")
        dbg_gidu_d = nc.dram_tensor("dbg_gidu", [128, N_GATH], U32,
                                    kind="ExternalOutput")
        dbg_s128_d = nc.dram_tensor("dbg_s128", [128, N_GATH], F32,
                                    kind="ExternalOutput")
        dbg_pld_d = nc.dram_tensor("dbg_pld", [128, 256], F32,
                                   kind="ExternalOutput")

    with tile.TileContext(nc) as tc:
        with tc.tile_pool(name="const", bufs=1) as cpool, \
             tc.tile_pool(name="qp", bufs=1) as qpool, \
             tc.tile_pool(name="rhs", bufs=4) as rpool, \
             tc.tile_pool(name="hc", bufs=3) as hpool, \
             tc.tile_pool(name="fold", bufs=3) as fpool, \
             tc.tile_pool(name="cand", bufs=2) as candp, \
             tc.tile_pool(name="fin", bufs=2) as finp, \
             tc.tile_pool(name="gath", bufs=2) as gpool, \
             tc.tile_pool(name="prod", bufs=2) as ppool, \
             tc.tile_pool(name="acc", bufs=1) as accp, \
             tc.tile_pool(name="ps", bufs=4, space="PSUM") as psp:

            # ---- constants ----
            # per-head fold-group member offsets {0,256,...,1792} + h*N_PAD
            ioff_h = []
            for h in range(N_HEADS):
                t = cpool.tile([128, 8], F32, tag=f"ioff{h}")
                nc.gpsimd.iota(t[:], pattern=[[256, 8]], base=h * N_PAD,
                               channel_multiplier=0,
                               allow_small_or_imprecise_dtypes=True)
                ioff_h.append(t)
            # p*512 offset per candidate position (pair-scan slots)
            joff = cpool.tile([128, N_PAIRS, 8], F32, tag="joff")
            nc.gpsimd.iota(joff[:], pattern=[[512, N_PAIRS], [0, 8]], base=0,
                           channel_multiplier=0,
                           allow_small_or_imprecise_dtypes=True)
            neg_inf8 = cpool.tile([128, 8], F32, tag="neg_inf8")
            nc.vector.memset(neg_inf8[:], -1e30)

            # ---- query tiles (all m upfront; small) ----
            q_s = []
            aux_s = []
            for m in range(M_TILES):
                qt = qpool.tile([KC, 128], BF16, tag=f"q{m}")
                nc.sync.dma_start(out=qt[:], in_=qpack_d[m])
                q_s.append(qt)
                at = qpool.tile([128, AUXW], F32, tag=f"aux{m}")
                nc.sync.dma_start(out=at[:], in_=aux_d[m])
                aux_s.append(at)

            numneg_all = accp.tile([128, M_TILES], F32, tag="numneg")
            wsum_all = accp.tile([128, M_TILES], F32, tag="wsum")

            for h in range(N_HEADS):
                # per-m-tile candidate stores for both endpoints
                cv = []
                cs = []
                for e in range(2):
                    cvt = candp.tile([128, N_PAIRS, 8], F16, tag=f"cv{e}")
                    cv.append(cvt)
                    cst = candp.tile([128, N_PAIRS, 8], U32, tag=f"cs{e}")
                    cs.append(cst)

                pldpair = None
                for j in range(N_SUPERS):
                    jpar = j % 2
                    p = j // 2
                    rhs_s = rpool.tile([KC, SUP], BF16, tag="rhs")
                    nc.sync.dma_start(
                        out=rhs_s[:], in_=rhs_d[h, :, j * SUP:(j + 1) * SUP])
                    if jpar == 0:
                        pldpair = fpool.tile([128, 2, 2, 256], F16, tag="pld")
                    hcv = hpool.tile([128, 2, SUP], F16, tag="hc")
                    for e in range(2):
                        m = 2 * h + e
                        for half in range(2):
                            psum = psp.tile([128, 1024], F32, tag="ps")
                            for b in range(2):
                                sl = slice(b * 512, (b + 1) * 512)
                                gl = slice(half * 1024 + b * 512,
                                           half * 1024 + (b + 1) * 512)
                                nc.tensor.matmul(psum[:, sl], q_s[m][:],
                                                 rhs_s[:, gl],
                                                 start=True, stop=True)
                            nc.scalar.activation(
                                hcv[:, e, half * 1024:(half + 1) * 1024],
                                psum[:],
                                mybir.ActivationFunctionType.Copy,
                                bias=0.0, scale=1.0)

                    # joint folds over both endpoints (fp16 2x mode)
                    f1 = fpool.tile([128, 2, 1024], F16, tag="f1")
                    nc.vector.tensor_tensor(
                        out=f1[:], in0=hcv[:, :, 0:1024],
                        in1=hcv[:, :, 1024:2048], op=mybir.AluOpType.max)
                    f2 = fpool.tile([128, 2, 512], F16, tag="f2")
                    nc.vector.tensor_tensor(
                        out=f2[:], in0=f1[:, :, 0:512], in1=f1[:, :, 512:1024],
                        op=mybir.AluOpType.max)
                    nc.vector.tensor_tensor(
                        out=pldpair[:, :, jpar], in0=f2[:, :, 0:256],
                        in1=f2[:, :, 256:512], op=mybir.AluOpType.max)

                    # scans per super-pair (global slot space j*256+f)
                    if jpar == 1 or j == N_SUPERS - 1:
                        for e in range(2):
                            if jpar == 1:
                                pin = pldpair[:, e].rearrange("p a b -> p (a b)")
                            else:
                                pin = pldpair[:, e, 0]
                            nc.vector.max(cv[e][:, p], pin)
                            nc.vector.max_index(cs[e][:, p], cv[e][:, p], pin)

                for e in range(2):
                    m = 2 * h + e
                    qn_s = aux_s[m][:, 0:1]
                    qf_s = aux_s[m][:, 1:2]
                    f_row = aux_s[m][:, 2:2 + DIM]
                    q_row = aux_s[m][:, 2 + DIM:2 + 2 * DIM]

                    # ---- pack candidates: qv*8192 + gslot ----
                    cvf = finp.tile([128, N_CAND], F32, tag="cvf")
                    # qv = round(clip((v+42)*32, 0, 2047))
                    nc.gpsimd.tensor_scalar(
                        out=cvf[:], in0=cv[e][:].rearrange("p a b -> p (a b)"),
                        scalar1=32.0, scalar2=None,
                        op0=mybir.AluOpType.mult)
                    nc.gpsimd.tensor_scalar(
                        out=cvf[:], in0=cvf[:], scalar1=1344.0,
                        scalar2=TWO23, op0=mybir.AluOpType.add,
                        op1=mybir.AluOpType.add)
                    nc.gpsimd.tensor_scalar(
                        out=cvf[:], in0=cvf[:], scalar1=TWO23,
                        scalar2=None, op0=mybir.AluOpType.subtract)
                    nc.gpsimd.tensor_scalar(
                        out=cvf[:], in0=cvf[:], scalar1=2047.0,
                        scalar2=0.0, op0=mybir.AluOpType.min,
                        op1=mybir.AluOpType.max)
                    slotf = finp.tile([128, N_CAND], F32, tag="slotf")
                    nc.vector.tensor_copy(
                        slotf[:], cs[e][:].rearrange("p a b -> p (a b)"))
                    gslot = finp.tile([128, N_CAND], F32, tag="gslot")
                    nc.vector.tensor_tensor(
                        out=gslot[:], in0=slotf[:],
                        in1=joff[:].rearrange("p a b -> p (a b)"),
                        op=mybir.AluOpType.add)
                    packed = finp.tile([128, N_CAND], F32, tag="packed")
                    nc.vector.scalar_tensor_tensor(
                        out=packed[:], in0=cvf[:], scalar=8192.0,
                        in1=gslot[:], op0=mybir.AluOpType.mult,
                        op1=mybir.AluOpType.add)
                    if debug and m == 0:
                        cvdbg = finp.tile([128, N_CAND], F32, tag="cvdbg")
                        nc.vector.tensor_copy(
                            cvdbg[:], cv[e][:].rearrange("p a b -> p (a b)"))
                        nc.sync.dma_start(out=dbg_cv_d[:], in_=cvdbg[:])
                        csdbg = finp.tile([128, N_CAND], U32, tag="csdbg")
                        nc.vector.tensor_copy(
                            csdbg[:], cs[e][:].rearrange("p a b -> p (a b)"))
                        nc.sync.dma_start(out=dbg_cs_d[:], in_=csdbg[:])
                        nc.sync.dma_start(out=dbg_pk_d[:], in_=packed[:])

                    # ---- top-16 packed (ties impossible) ----
                    w16 = finp.tile([128, N_WIN], F32, tag="w16")
                    nc.vector.max(w16[:, 0:8], packed[:])
                    prep = finp.tile([128, N_CAND], F32, tag="prep")
                    nc.vector.match_replace(prep[:], w16[:, 0:8], packed[:],
                                            -1e30)
                    nc.vector.max(w16[:, 8:16], prep[:])

                    # ---- decode: gslot16 = w16 mod 8192 -> (j, f) -> base ----
                    qv16 = finp.tile([128, N_WIN], F32, tag="qv16")
                    nc.gpsimd.tensor_scalar(
                        out=qv16[:], in0=w16[:], scalar1=1.0 / 8192.0,
                        scalar2=None, op0=mybir.AluOpType.mult)
                    nc.vector.tensor_scalar(
                        out=qv16[:], in0=qv16[:], scalar1=-0.49,
                        scalar2=TWO23, op0=mybir.AluOpType.add,
                        op1=mybir.AluOpType.add)
                    nc.vector.tensor_scalar(
                        out=qv16[:], in0=qv16[:], scalar1=TWO23,
                        scalar2=None, op0=mybir.AluOpType.subtract)
                    g16 = finp.tile([128, N_WIN], F32, tag="g16")
                    nc.vector.scalar_tensor_tensor(
                        out=g16[:], in0=qv16[:], scalar=-8192.0,
                        in1=w16[:], op0=mybir.AluOpType.mult,
                        op1=mybir.AluOpType.add)
                    # j16 = floor(g16/256); f16 = g16 - 256*j16
                    j16 = finp.tile([128, N_WIN], F32, tag="j16")
                    nc.gpsimd.tensor_scalar(
                        out=j16[:], in0=g16[:], scalar1=1.0 / 256.0,
                        scalar2=None, op0=mybir.AluOpType.mult)
                    nc.vector.tensor_scalar(
                        out=j16[:], in0=j16[:], scalar1=-0.498046875,
                        scalar2=TWO23, op0=mybir.AluOpType.add,
                        op1=mybir.AluOpType.add)
                    nc.vector.tensor_scalar(
                        out=j16[:], in0=j16[:], scalar1=TWO23,
                        scalar2=None, op0=mybir.AluOpType.subtract)
                    # base16 = g16 + 1792*j16  (= j*2048 + f)
                    base16 = finp.tile([128, N_WIN], F32, tag="base16")
                    nc.vector.scalar_tensor_tensor(
                        out=base16[:], in0=j16[:], scalar=1792.0,
                        in1=g16[:], op0=mybir.AluOpType.mult,
                        op1=mybir.AluOpType.add)
                    # gid128 = base16 + {0,256,...,1792} + h*N_PAD
                    gidf = finp.tile([128, N_WIN, 8], F32, tag="gidf")
                    nc.vector.tensor_tensor(
                        out=gidf[:],
                        in0=base16[:].rearrange("p (a b) -> p a b", b=1)
                            .to_broadcast((128, N_WIN, 8)),
                        in1=ioff_h[h][:].rearrange("p (a b) -> p a b", a=1)
                            .to_broadcast((128, N_WIN, 8)),
                        op=mybir.AluOpType.add)
                    gidu = finp.tile([128, N_GATH], U32, tag="gidu")
                    nc.vector.tensor_copy(
                        gidu[:], gidf[:].rearrange("p a b -> p (a b)"))
                    if debug and m == 0:
                        nc.sync.dma_start(out=dbg_w16_d[:], in_=w16[:])
                        nc.sync.dma_start(out=dbg_gidu_d[:], in_=gidu[:])

                    # ---- gather the 16 winner groups (272 B rows) ----
                    goff = finp.tile([128, N_WIN], F32, tag="goff")
                    nc.vector.tensor_scalar(
                        out=goff[:], in0=g16[:],
                        scalar1=float(h * N_SUPERS * 256), scalar2=None,
                        op0=mybir.AluOpType.add)
                    goffu = finp.tile([128, N_WIN], U32, tag="goffu")
                    nc.vector.tensor_copy(goffu[:], goff[:])
                    gath = gpool.tile([128, N_WIN, 8, EW], F32, tag="gath")
                    for w in range(N_WIN):
                        nc.gpsimd.indirect_dma_start(
                            out=gath[:, w].rearrange("p k d -> p (k d)"),
                            out_offset=None,
                            in_=embg_d[:],
                            in_offset=IndirectOffsetOnAxis(
                                ap=goffu[:, w:w + 1], axis=0))

                    # ---- exact recompute s = 2 q.e - (qn + en) ----
                    prod = ppool.tile([128, N_WIN, 8, DIM], F32, tag="prod")
                    for dd in range(DIM):
                        nc.gpsimd.tensor_scalar(
                            out=prod[:, :, :, dd:dd + 1],
                            in0=gath[:, :, :, dd:dd + 1],
                            scalar1=q_row[:, dd:dd + 1], scalar2=None,
                            op0=mybir.AluOpType.mult)
                    dot = finp.tile([128, N_GATH], F32, tag="dot")
                    nc.vector.tensor_reduce(dot[:], prod[:],
                                            axis=mybir.AxisListType.X,
                                            op=mybir.AluOpType.add)
                    t128 = finp.tile([128, N_GATH], F32, tag="t128")
                    nc.vector.tensor_scalar(
                        out=t128[:],
                        in0=gath[:, :, :, DIM:DIM + 1]
                            .rearrange("p w k o -> p (w k o)"),
                        scalar1=qn_s, scalar2=None,
                        op0=mybir.AluOpType.add)
                    s128 = finp.tile([128, N_GATH], F32, tag="s128")
                    nc.vector.scalar_tensor_tensor(
                        out=s128[:], in0=dot[:], scalar=2.0, in1=t128[:],
                        op0=mybir.AluOpType.mult,
                        op1=mybir.AluOpType.subtract)
                    if debug and m == 0:
                        nc.sync.dma_start(out=dbg_s128_d[:], in_=s128[:])

                    # ---- exact top-9, drop rank-1 (self) ----
                    m1 = finp.tile([128, 1], F32, tag="m1")
                    nc.vector.tensor_reduce(m1[:], s128[:],
                                            axis=mybir.AxisListType.X,
                                            op=mybir.AluOpType.max)
                    m1x8 = finp.tile([128, 8], F32, tag="m1x8")
                    nc.vector.tensor_copy(m1x8[:], neg_inf8[:])
                    nc.vector.tensor_copy(m1x8[:, 0:1], m1[:])
                    srep = finp.tile([128, N_GATH], F32, tag="srep")
                    nc.vector.match_replace(srep[:], m1x8[:], s128[:], -1e30)
                    w8 = finp.tile([128, 8], F32, tag="w8")
                    nc.vector.max(w8[:], srep[:])
                    srep2 = finp.tile([128, N_GATH], F32, tag="srep2")
                    nc.vector.match_replace(srep2[:], w8[:], srep[:], 1e30)
                    mask = finp.tile([128, N_GATH], F32, tag="mask")
                    nc.vector.tensor_scalar(out=mask[:], in0=srep2[:],
                                            scalar1=1e29, scalar2=None,
                                            op0=mybir.AluOpType.is_ge)
                    gidsel = finp.tile([128, N_GATH], F32, tag="gidsel")
                    nc.vector.scalar_tensor_tensor(
                        out=gidsel[:], in0=gidf[:].rearrange("p a b -> p (a b)"),
                        scalar=1.0, in1=mask[:],
                        op0=mybir.AluOpType.add, op1=mybir.AluOpType.mult)
                    wgidf = finp.tile([128, 8], F32, tag="wgidf")
                    nc.vector.max(wgidf[:], gidsel[:])
                    wgidu = finp.tile([128, 8], U32, tag="wgidu")
                    nc.vector.tensor_scalar(
                        out=wgidu[:], in0=wgidf[:], scalar1=-1.0,
                        scalar2=None, op0=mybir.AluOpType.add)
                    nc.sync.dma_start(out=dbg_gid_d[m], in_=wgidu[:])
                    nc.sync.dma_start(out=dbg_s_d[m], in_=w8[:])

                    # ---- gather the 8 winners, field dots, weights ----
                    g2 = gpool.tile([128, 8, EW], F32, tag="g2")
                    for k in range(8):
                        nc.gpsimd.indirect_dma_start(
                            out=g2[:, k], out_offset=None,
                            in_=embn_d[:],
                            in_offset=IndirectOffsetOnAxis(
                                ap=wgidu[:, k:k + 1], axis=0))
                    prod8 = finp.tile([128, 8, DIM], F32, tag="prod8")
                    nc.vector.tensor_tensor(
                        out=prod8[:], in0=g2[:, :, 0:DIM],
                        in1=q_row.rearrange("p (o d) -> p o d", o=1)
                            .to_broadcast((128, 8, DIM)),
                        op=mybir.AluOpType.mult)
                    dot8 = finp.tile([128, 8], F32, tag="dot8")
                    nc.vector.tensor_reduce(dot8[:], prod8[:],
                                            axis=mybir.AxisListType.X,
                                            op=mybir.AluOpType.add)
                    t8 = finp.tile([128, 8], F32, tag="t8")
                    nc.vector.tensor_scalar(out=t8[:], in0=g2[:, :, DIM],
                                            scalar1=qn_s, scalar2=None,
                                            op0=mybir.AluOpType.add)
                    s8 = finp.tile([128, 8], F32, tag="s8")
                    nc.vector.scalar_tensor_tensor(
                        out=s8[:], in0=dot8[:], scalar=2.0, in1=t8[:],
                        op0=mybir.AluOpType.mult,
                        op1=mybir.AluOpType.subtract)
                    nc.vector.tensor_scalar(out=s8[:], in0=s8[:], scalar1=0.0,
                                            scalar2=None,
                                            op0=mybir.AluOpType.min)
                    dist8 = finp.tile([128, 8], F32, tag="dist8")
                    nc.scalar.activation(dist8[:], s8[:],
                                         mybir.ActivationFunctionType.Sqrt,
                                         bias=0.0, scale=-1.0)
                    wexp8 = finp.tile([128, 8], F32, tag="wexp8")
                    nc.scalar.activation(wexp8[:], dist8[:],
                                         mybir.ActivationFunctionType.Exp,
                                         bias=1.0, scale=-1.0)
                    prodf8 = finp.tile([128, 8, DIM], F32, tag="prodf8")
                    nc.vector.tensor_tensor(
                        out=prodf8[:], in0=g2[:, :, 0:DIM],
                        in1=f_row.rearrange("p (o d) -> p o d", o=1)
                            .to_broadcast((128, 8, DIM)),
                        op=mybir.AluOpType.mult)
                    u8 = finp.tile([128, 8], F32, tag="u8")
                    nc.vector.tensor_reduce(u8[:], prodf8[:],
                                            axis=mybir.AxisListType.X,
                                            op=mybir.AluOpType.add)
                    scrap8 = finp.tile([128, 8], F32, tag="scrap8")
                    nc.vector.scalar_tensor_tensor(
                        out=scrap8[:], in0=u8[:], scalar=qf_s, in1=wexp8[:],
                        op0=mybir.AluOpType.subtract,
                        op1=mybir.AluOpType.mult,
                        accum_out=numneg_all[:, m:m + 1])
                    nc.vector.tensor_reduce(wsum_all[:, m:m + 1], wexp8[:],
                                            axis=mybir.AxisListType.X,
                                            op=mybir.AluOpType.add)

            # ---- combine heads: pred = sigmoid(mean_h num_h / den_h) ----
            sp = finp
            nsum2 = sp.tile([128, N_HEADS], F32, tag="nsum2")
            nc.vector.tensor_reduce(
                nsum2[:], numneg_all[:].rearrange("p (h e) -> p h e", e=2),
                axis=mybir.AxisListType.X, op=mybir.AluOpType.add)
            den = sp.tile([128, N_HEADS], F32, tag="den")
            nc.vector.tensor_reduce(
                den[:], wsum_all[:].rearrange("p (h e) -> p h e", e=2),
                axis=mybir.AxisListType.X, op=mybir.AluOpType.add)
            den8 = sp.tile([128, N_HEADS], F32, tag="den8")
            nc.vector.tensor_scalar(out=den8[:], in0=den[:],
                                    scalar1=float(N_SENT), scalar2=None,
                                    op0=mybir.AluOpType.add)
            rden = sp.tile([128, N_HEADS], F32, tag="rden")
            nc.vector.reciprocal(rden[:], den8[:])
            ratio = sp.tile([128, N_HEADS], F32, tag="ratio")
            nc.vector.tensor_tensor(out=ratio[:], in0=nsum2[:], in1=rden[:],
                                    op=mybir.AluOpType.mult)
            ssum = sp.tile([128, 1], F32, tag="ssum")
            nc.vector.tensor_reduce(ssum[:], ratio[:],
                                    axis=mybir.AxisListType.X,
                                    op=mybir.AluOpType.add)
            preds_s = sp.tile([128, 1], F32, tag="preds")
            nc.scalar.activation(preds_s[:], ssum[:],
                                 mybir.ActivationFunctionType.Sigmoid,
                                 bias=0.0, scale=-1.0 / N_HEADS)
            nc.sync.dma_start(out=preds_d[:], in_=preds_s[:])

    return nc


def _prep_inputs(embeds, field, edges):
    """Host-side layout prep + per-core sharding."""
    embeds = np.asarray(embeds, dtype=np.float32)
    field = np.asarray(field, dtype=np.float32)
    edges = np.asarray(edges)

    import ml_dtypes
    bf16 = ml_dtypes.bfloat16
    en = np.sum(np.square(embeds), axis=-1, dtype=np.float32)
    en_pad = np.full((N_HEADS, N_PAD), 60000.0, np.float32)
    en_pad[:, :N_NODES] = en
    en_hi = en_pad.astype(bf16).astype(np.float32)
    en_lo = en_pad - en_hi
    rhs_aug = np.zeros((N_HEADS, KC, N_PAD), dtype=bf16)
    rhs_aug[:, :DIM, :N_NODES] = embeds.transpose(0, 2, 1).astype(bf16)
    rhs_aug[:, DIM, :] = en_hi.astype(bf16)
    rhs_aug[:, DIM + 1, :] = en_lo.astype(bf16)

    embn = np.zeros((N_HEADS * N_PAD, EW), dtype=np.float32)
    embn3 = embn.reshape(N_HEADS, N_PAD, EW)
    embn3[:, :N_NODES, :DIM] = embeds
    embn3[:, :N_NODES, DIM] = en
    embn3[:, N_NODES:, DIM] = 60000.0

    # group-major table: row (h, j, f) = members {j*2048 + f + 256k}
    embg = np.ascontiguousarray(
        embn3.reshape(N_HEADS, N_SUPERS, 8, 256, EW)
             .transpose(0, 1, 3, 2, 4)
             .reshape(N_HEADS * N_SUPERS * 256, 8 * EW))

    in_maps = []
    for c in range(N_CORES):
        sl = slice(c * EDGES_PER_CORE, (c + 1) * EDGES_PER_CORE)
        qpack = np.zeros((M_TILES, KC, 128), dtype=bf16)
        aux = np.zeros((M_TILES, 128, AUXW), dtype=np.float32)
        for m in range(M_TILES):
            h, e = m // 2, m % 2
            nodes = edges[e, sl]
            q = embeds[h, nodes]                      # (128, 32)
            f = field[h, nodes]                       # (128, 32)
            qpack[m, :DIM] = (2.0 * q).T.astype(bf16)
            qpack[m, DIM] = -1.0
            qpack[m, DIM + 1] = -1.0
            aux[m, :, 0] = np.einsum('bd,bd->b', q, q)
            aux[m, :, 1] = np.einsum('bd,bd->b', q, f)
            aux[m, :, 2:2 + DIM] = f
            aux[m, :, 2 + DIM:] = q
        in_maps.append({
            "rhs_aug": rhs_aug, "embn": embn, "embg": embg,
            "qpack": qpack, "aux": aux,
        })
    return in_maps


def kernel(embeds, field, edges):
    from concourse.bass_utils import run_bass_kernel_spmd

    nc = _build_program()
    nc.finalize()
    in_maps = _prep_inputs(embeds, field, edges)
    core_ids = list(range(N_CORES))
    trace = bool(os.environ.get("KNN_TRACE"))
    tmpdir = os.environ.get("KNN_TRACE_DIR") or None
    out = run_bass_kernel_spmd(nc, in_maps, core_ids, trace=trace,
                               tmpdir=tmpdir)
    LAST["results"] = out
    preds = np.concatenate(
        [out.results[c]["preds"][:, 0] for c in range(N_CORES)])
    return preds.astype(np.float32)
